# revision 6
# baseline (speedup 1.0000x reference)
"""MoE 2D router kernel for 8 Trainium2 NeuronCores — v2.

Strategy (pure data parallel, batch-sharded, 2 batches/core):
  - Layout: per batch [C=16, H*W=16384] viewed as [128, 2048] in SBUF with
    partition p = c*8 + blk (blk = pixel-block of 2048 contiguous pixels):
    channel params are per-partition scalars, HBM lines fully contiguous.
  - Expert-axis work done in T-space (PE f32 transpose of 128-col blocks,
    bit-exact): per-pixel m1/m2/softmax-sum are free-axis strided reduces to
    [128, 32] scalars; broadcasts back over the expert axis are free-axis
    stride-0 APs (no selection matmuls).  max_excl and the hard-softmax G
    are built in T-space and PE-transposed back to A-space.
  - GpSimd cannot touch PSUM, so hlT is copied PSUM->SBUF once per chunk by
    the ACT engine (Copy lives in every activation table: no table load);
    the two PSUM-consuming stt ops (numer) and PSUM copies (gs) go to DVE.
  - G = mask * exp(m1)/sum(exp(hl)): exp(m1) hits the same Exp table entry
    as exp(hl[argmax]) would, so this is bit-identical to masked softmax.
  - softplus(t) = Ln(1 + Exp(t)); erf from the direct Erf table entry.
    Activation ops are emitted in grouped phases (all Exp, all Ln, all
    Exp, all Erf) so the per-function tables load only ~4 times/exec.
  - The erf argument divides by wnoise via one DVE reciprocal + a Pool
    multiply (no divide opcode exists on the vector engines).  Elementwise
    work is split DVE/Pool/ACT to balance all three near the DMA roofline:
    per-execution engine busy is ~ACT 29us / DVE 28us / Pool 27us / DMA
    25us / PE 12us (cost model: 53us single-shot, ~40us steady-state,
    vs 104us for the selection-matmul baseline).
  - _build(n_reps=N) replicates the body N times back-to-back for the
    loop-timing NEFF used by test.py (production kernel() uses n_reps=1;
    the hw_loop flag wraps the body in a For_i hardware loop instead, but
    its build time is prohibitive with the current Tile control-flow
    tracker, so test.py uses the replicated form).
"""
import sys

sys.path.insert(0, "/opt/trn_rl_repo")

import numpy as np

B, C, H, W = 16, 16, 128, 128
NCORES = 8
BPC = B // NCORES           # batches per core
HW = H * W                  # 16384 pixels per (batch, channel)
NBLK = 8                    # pixel blocks per batch (HW / 2048)
FB = C * HW // 128          # free size per batch in [128, FB] layout = 2048
NCH = 4                     # 128-col groups per chunk
CHW = 512                   # chunk width
VB = 4                      # virtual pipeline batches per core
FBV = BPC * FB // VB        # free size per virtual batch = 1024
NCHV = FBV // CHW           # chunks per virtual batch = 2

_CACHE = {}

# Newton-refine the Exp output through the Ln table before the softplus Ln
# (extra ACT pass + two fused vector passes; only needed if the raw Exp
# table error visibly degrades the load_loss erf path).
REFINE = False

# Engine assignment knobs for the balance-tunable elementwise ops.
# (GpSimd cannot read PSUM and has no scalar-tensor-tensor opcode, so hl /
# numer / gs are DVE-bound; mkB is an immediate-scalar scale.)
ENG = {
    "nw": "pool", "mk": "dve", "mkB": "pool", "md": "pool",
    "t1": "pool", "mex": "pool", "gT": "pool",
    "numer": "dve", "gv": "pool", "m2n": "pool",
}


def _build(n_reps=1, hw_loop=False):
    import concourse.bacc as bacc
    import concourse.mybir as mybir
    from concourse.tile import TileContext, add_dep_helper

    f32 = mybir.dt.float32
    AX = mybir.AxisListType
    OP = mybir.AluOpType
    AF = mybir.ActivationFunctionType
    BIGNEG = -1e30

    nc = bacc.Bacc(trn_type="TRN2", target_bir_lowering=False, debug=False,
                   num_devices=NCORES,
                   name=f"moe_router_v2_r{n_reps}{'h' if hw_loop else ''}")

    def eng(op):
        return {"pool": nc.gpsimd, "dve": nc.vector}[ENG[op]]

    xd = nc.dram_tensor("x", [BPC, 128, FB], f32, kind="ExternalInput")
    nd = nc.dram_tensor("noise", [BPC, 128, FB], f32, kind="ExternalInput")
    wgp_d = nc.dram_tensor("wgp", [128, 1], f32, kind="ExternalInput")
    wnp_d = nc.dram_tensor("wnp", [128, 1], f32, kind="ExternalInput")
    id_f = nc.dram_tensor("id_f", [128, 128], f32, kind="ExternalInput")
    gd = nc.dram_tensor("g_out", [BPC, 128, FB], f32, kind="ExternalOutput")
    ld = nc.dram_tensor("load_out", [BPC, 128, FB], f32, kind="ExternalOutput")

    def rT(t, order="p (g c k) -> p g c k"):
        return t.rearrange(order, g=NCH, c=C)

    def rB(t):
        # [128, 32] (g, k) per-pixel scalar -> broadcast over the c axis
        return (t.rearrange("p (g k) -> p g k", g=NCH)
                .unsqueeze(2).broadcast_to([128, NCH, C, NBLK]))

    with TileContext(nc) as tc:
        with tc.tile_pool(name="const", bufs=1) as cpool, \
             tc.tile_pool(name="vb", bufs=1) as vbp, \
             tc.tile_pool(name="xin", bufs=2) as xinp, \
             tc.tile_pool(name="io", bufs=2) as iop, \
             tc.tile_pool(name="chunk", bufs=3) as chp, \
             tc.tile_pool(name="sc", bufs=4) as scp, \
             tc.tile_pool(name="ps_hl", bufs=2, space="PSUM") as ps_hl, \
             tc.tile_pool(name="ps_mex", bufs=2, space="PSUM") as ps_mex, \
             tc.tile_pool(name="ps_g", bufs=2, space="PSUM") as ps_g:

            consts_loaded = [None]

            def _load_consts():
                wgp = cpool.tile([128, 1], f32, tag="wgp")
                nc.sync.dma_start(out=wgp[:, :], in_=wgp_d[:, :])
                wnp = cpool.tile([128, 1], f32, tag="wnp")
                nc.sync.dma_start(out=wnp[:, :], in_=wnp_d[:, :])
                idf = cpool.tile([128, 128], f32, tag="idf")
                nc.sync.dma_start(out=idf[:, :], in_=id_f[:, :])
                return wgp, wnp, idf

            prev_erf = [None]

            def _emit_rep():
                xts, nts, eu0s, wns, hls, qts = {}, {}, {}, {}, {}, {}

                # -- phase 1: input DMA + Exp(x*wnp)  [Exp table] --
                last_exp_inst = [None]
                for b in range(VB):
                    bb, bo = divmod(b, VB // BPC)
                    bs = bo * FBV
                    xt = xinp.tile([128, FBV], f32, tag=f"x{b}")
                    nc.sync.dma_start(out=xt[:, :], in_=xd[bb, :, bs:bs + FBV])
                    nt = xinp.tile([128, FBV], f32, tag=f"n{b}")
                    nc.sync.dma_start(out=nt[:, :], in_=nd[bb, :, bs:bs + FBV])
                    if consts_loaded[0] is None:
                        consts_loaded[0] = _load_consts()
                    wgp, wnp, idf = consts_loaded[0]
                    eu0 = vbp.tile([128, FBV], f32, tag=f"eu0{b}")
                    eu0_inst = nc.scalar.activation(
                        eu0[:, :], xt[:, :], AF.Exp, scale=wnp[:, :])
                    last_exp_inst[0] = eu0_inst
                    xts[b], nts[b], eu0s[b] = xt, nt, eu0

                # -- phase 2: softplus Ln (+ optional Newton)  [Ln table] --
                if REFINE:
                    lcs = {}
                    for b in range(VB):
                        lc = vbp.tile([128, FBV], f32, tag=f"lc{b}")
                        lc_inst = nc.scalar.activation(lc[:, :],
                                                       eu0s[b][:, :], AF.Ln)
                        lcs[b] = lc
                    for b in range(VB):
                        wgp, wnp, idf = consts_loaded[0]
                        d2 = vbp.tile([128, FBV], f32, tag=f"lc{b}")
                        nc.vector.scalar_tensor_tensor(
                            d2[:, :], xts[b][:, :], wnp[:, :],
                            lcs[b][:, :], op0=OP.mult, op1=OP.subtract)
                        eu = vbp.tile([128, FBV], f32, tag=f"eu0{b}")
                        nc.gpsimd.scalar_tensor_tensor(
                            eu[:, :], d2[:, :], 1.0, eu0s[b][:, :],
                            op0=OP.add, op1=OP.mult)
                        eu0s[b] = eu
                last_ln_inst = [None]
                for b in range(VB):
                    wgp, wnp, idf = consts_loaded[0]
                    wn = vbp.tile([128, FBV], f32, tag=f"wn{b}")
                    ln_inst = nc.scalar.activation(wn[:, :], eu0s[b][:, :],
                                                   AF.Ln, bias=1.0)
                    last_ln_inst[0] = ln_inst
                    nw = vbp.tile([128, FBV], f32, tag=f"eu0{b}")
                    eng("nw").tensor_tensor(nw[:, :], nts[b][:, :], wn[:, :],
                                            op=OP.mult)
                    xw = vbp.tile([128, FBV], f32, tag=f"q{b}")
                    nc.gpsimd.tensor_scalar_mul(xw[:, :], xts[b][:, :],
                                                wgp[:, :])
                    hl = vbp.tile([128, FBV], f32, tag=f"hl{b}")
                    nc.gpsimd.tensor_tensor(hl[:, :], xw[:, :], nw[:, :],
                                            op=OP.add)
                    hls[b] = hl
                    # 1/wnoise for the erf argument (no divide opcode on the
                    # vector engines; reciprocal + Pool multiply instead)
                    rw = vbp.tile([128, FBV], f32, tag=f"eu0{b}")
                    nc.vector.reciprocal(rw[:, :], wn[:, :])
                    wns[b] = rw

                # -- phase 3: per-chunk T-space routing  [Exp table] --
                last_a_inst = [None]
                for b in range(VB):
                    bb, bo = divmod(b, VB // BPC)
                    bs = bo * FBV
                    wgp, wnp, idf = consts_loaded[0]
                    xt, hl, rw = xts[b], hls[b], wns[b]
                    qt = vbp.tile([128, FBV], f32, tag=f"q{b}")
                    for ch in range(NCHV):
                        cs = ch * CHW
                        hlT = ps_hl.tile([128, CHW], f32, tag="hlT")
                        for g in range(NCH):
                            nc.tensor.transpose(
                                hlT[:, g * 128:(g + 1) * 128],
                                hl[:, cs + g * 128:cs + (g + 1) * 128],
                                idf[:, :])
                        # one PSUM->SBUF copy so GpSimd ops can see hl in T
                        hlS = chp.tile([128, CHW], f32, tag="hlS")
                        last_a_inst[0] = nc.scalar.activation(
                            hlS[:, :], hlT[:, :], AF.Copy)
                        m1c = scp.tile([128, 32], f32, tag="m1c")
                        nc.vector.tensor_reduce(
                            m1c[:, :], rT(hlS[:, :], "p (g c k) -> p g k c"),
                            axis=AX.X, op=OP.max)
                        m1b = rB(m1c[:, :])
                        # exact argmax mask (hlS and m1c share the same bits)
                        mk = chp.tile([128, CHW], f32, tag="mk")
                        eng("mk").tensor_tensor(rT(mk[:, :]), rT(hlS[:, :]),
                                                m1b, op=OP.is_equal)
                        # second max: mask argmax to -inf, re-reduce
                        mkB = chp.tile([128, CHW], f32, tag="mkB")
                        eng("mkB").tensor_scalar_mul(mkB[:, :], mk[:, :],
                                                     BIGNEG)

                        md = chp.tile([128, CHW], f32, tag="md")
                        eng("md").tensor_tensor(md[:, :], mkB[:, :],
                                                hlS[:, :], op=OP.add)
                        m2c = scp.tile([128, 32], f32, tag="m2c")
                        nc.vector.tensor_reduce(
                            m2c[:, :], rT(md[:, :], "p (g c k) -> p g k c"),
                            axis=AX.X, op=OP.max)
                        m2n = scp.tile([128, 32], f32, tag="m2n")
                        eng("m2n").tensor_tensor(m2n[:, :], m2c[:, :],
                                                 m1c[:, :], op=OP.subtract)
                        # softmax pieces: ssum, exp(m1), gv = exp(m1)/ssum
                        expT = chp.tile([128, CHW], f32, tag="expT")
                        exp_inst = nc.scalar.activation(
                            expT[:, :], hlS[:, :], AF.Exp)
                        last_a_inst[0] = exp_inst
                        ssum = scp.tile([128, 32], f32, tag="ssum")
                        nc.vector.tensor_reduce(
                            ssum[:, :], rT(expT[:, :], "p (g c k) -> p g k c"),
                            axis=AX.X, op=OP.add)
                        em1 = scp.tile([128, 32], f32, tag="em1")
                        em1_inst = nc.scalar.activation(
                            em1[:, :], m1c[:, :], AF.Exp)
                        last_a_inst[0] = em1_inst
                        srecip = scp.tile([128, 32], f32, tag="srecip")
                        nc.vector.reciprocal(srecip[:, :], ssum[:, :])
                        gv = scp.tile([128, 32], f32, tag="gv")
                        eng("gv").tensor_tensor(gv[:, :], em1[:, :],
                                                srecip[:, :], op=OP.mult)
                        # hard-softmax output in T-space
                        gT = chp.tile([128, CHW], f32, tag="gT")
                        eng("gT").tensor_tensor(rT(gT[:, :]), rT(mk[:, :]),
                                                rB(gv[:, :]), op=OP.mult)
                        # max_excl in T-space: m1 + mk*(m2-m1)
                        t1 = chp.tile([128, CHW], f32, tag="t1")
                        eng("t1").tensor_tensor(rT(t1[:, :]), rT(mk[:, :]),
                                                rB(m2n[:, :]), op=OP.mult)
                        mex = chp.tile([128, CHW], f32, tag="mex")
                        eng("mex").tensor_tensor(rT(mex[:, :]), rT(t1[:, :]),
                                                 m1b, op=OP.add)
                        # back to A-space
                        mexA = ps_mex.tile([128, CHW], f32, tag="mexA")
                        gA = ps_g.tile([128, CHW], f32, tag="gA")
                        for g in range(NCH):
                            nc.tensor.transpose(
                                mexA[:, g * 128:(g + 1) * 128],
                                mex[:, g * 128:(g + 1) * 128], idf[:, :])
                            nc.tensor.transpose(
                                gA[:, g * 128:(g + 1) * 128],
                                gT[:, g * 128:(g + 1) * 128], idf[:, :])
                        gs = chp.tile([128, CHW], f32, tag="gs")
                        nc.scalar.activation(gs[:, :], gA[:, :], AF.Copy)
                        nc.sync.dma_start(out=gd[bb, :, bs + cs:bs + cs + CHW],
                                          in_=gs[:, :])
                        # erf argument: q = (x*wgp - max_excl) / wnoise
                        numer = chp.tile([128, CHW], f32, tag="numer")
                        eng("numer").scalar_tensor_tensor(
                            numer[:, :], xt[:, cs:cs + CHW], wgp[:, :],
                            mexA[:, :], op0=OP.mult, op1=OP.subtract)
                        nc.gpsimd.tensor_tensor(qt[:, cs:cs + CHW],
                                                numer[:, :],
                                                rw[:, cs:cs + CHW],
                                                op=OP.mult)
                    qts[b] = qt

                # -- phase 4: erf tails  [sigmoid/Erf table] --
                for b in range(VB):
                    bb, bo = divmod(b, VB // BPC)
                    bs = bo * FBV
                    lt = iop.tile([128, FBV], f32, tag="load")
                    erf_inst = nc.scalar.activation(lt[:, :], qts[b][:, :],
                                                    AF.Erf)
                    if b == 0:
                        add_dep_helper(erf_inst.ins, last_a_inst[0].ins,
                                       sync=True,
                                       reason="group Erf after all Exp ops")
                    prev_erf[0] = erf_inst
                    nc.sync.dma_start(out=ld[bb, :, bs:bs + FBV], in_=lt[:, :])

            if hw_loop and n_reps > 1:
                _load_consts_now = _load_consts()
                consts_loaded[0] = _load_consts_now
                with tc.For_i(0, n_reps) as _i:
                    _emit_rep()
            else:
                for rep in range(n_reps):
                    _emit_rep()

    nc.compile()
    _fix_act_tables(nc, mybir)
    return nc


def _fix_act_tables(nc, mybir):
    """Retarget Exp/Ln activation-table loads to the combined
    natural_log_exp table, then drop loads that reload the already-active
    table.  Without this the default per-function placement emits a 1.3us
    table load at nearly every Exp<->Ln transition."""
    from concourse.hw_specs import get_activation_tables
    AFT = mybir.ActivationFunctionType
    tabs = list(get_activation_tables(nc.m.arch).items())
    union_id = None
    for i, (_, fs) in enumerate(tabs):
        if AFT.Exp in fs and AFT.Ln in fs:
            union_id = i
            break
    assert union_id is not None
    union_funcs = tabs[union_id][1]
    for blk in nc.m.functions[0].blocks:
        insts = blk.instructions
        loads = []
        for idx, inst in enumerate(insts):
            if isinstance(inst, mybir.InstLoadActFuncSet):
                loads.append((idx, inst))
        for li, (idx, load) in enumerate(loads):
            end = loads[li + 1][0] if li + 1 < len(loads) else len(insts)
            funcs = {i2.func for i2 in insts[idx + 1:end]
                     if isinstance(i2, mybir.InstActivation)}
            if funcs and funcs.issubset(union_funcs):
                load.act_func_set_id = union_id
        cur = None
        to_remove = []
        for inst in insts:
            if isinstance(inst, mybir.InstLoadActFuncSet):
                if inst.act_func_set_id == cur and not inst.has_wait():
                    to_remove.append(inst)
                else:
                    cur = inst.act_func_set_id
            elif isinstance(inst, mybir.InstActivation):
                assert inst.func in tabs[cur][1], (inst.func, cur)
        for inst in to_remove:
            insts.remove(inst)


def _consts():
    return {"id_f": np.eye(128, dtype=np.float32)}


def make_in_maps(x, noise, wg_param, wnoise_param):
    consts = _consts()
    wgp = np.repeat(np.ascontiguousarray(wg_param, dtype=np.float32).reshape(C),
                    8).reshape(128, 1)
    wnp = np.repeat(np.ascontiguousarray(wnoise_param, dtype=np.float32
                                         ).reshape(C), 8).reshape(128, 1)
    x = np.ascontiguousarray(x, dtype=np.float32)
    noise = np.ascontiguousarray(noise, dtype=np.float32)
    in_maps = []
    for i in range(NCORES):
        xs = x[i * BPC:(i + 1) * BPC].reshape(BPC, 128, FB)
        ns = noise[i * BPC:(i + 1) * BPC].reshape(BPC, 128, FB)
        in_maps.append({"x": xs, "noise": ns, "wgp": wgp, "wnp": wnp, **consts})
    return in_maps


def kernel(x, noise, wg_param, wnoise_param):
    from concourse.bass_utils import run_bass_kernel_spmd

    if "nc" not in _CACHE:
        _CACHE["nc"] = _build()
    nc = _CACHE["nc"]
    in_maps = make_in_maps(x, noise, wg_param, wnoise_param)
    res = run_bass_kernel_spmd(nc, in_maps, list(range(NCORES)))
    G = np.empty((B, C, H, W), dtype=np.float32)
    L = np.empty((B, C, H, W), dtype=np.float32)
    for i in range(NCORES):
        G[i * BPC:(i + 1) * BPC] = res.results[i]["g_out"].reshape(BPC, C, H, W)
        L[i * BPC:(i + 1) * BPC] = res.results[i]["load_out"].reshape(BPC, C, H, W)
    return G, L
